# revision 2
# baseline (speedup 1.0000x reference)
"""Trainium2 Bass kernel for nn_DotAttention (B=4, Tq=Tv=2048, D=1024, 16 heads).

Core c -> head pair (2c, 2c+1) = att-dim slice [128c, 128c+128) for ALL 4
batches (load-balanced across the per-batch masked lengths).

v2 changes over v1:
- Row-tiled energy: the two heads' energy matmuls run CONCURRENTLY as two
  K=64 row tiles of the PE array (tile_position (0,0)/(64,0)) instead of one
  zero-padded K=128 contraction per head. kT is the natural [128, Tv] layout
  (head0 dims on partitions 0-63, head1 on 64-127); no zero bands, no z64.
- Software-pipelined context: ctx matmuls for j-pair p are emitted after the
  energy+exp of pair p+1, so the PE FIFO never head-blocks on the exp (ACT)
  result or on the softmax-normalization chain of the previous ib block.
- The softmax chain (DVE recip -> gpsimd partition_broadcast -> mul) is
  followed by a large filler budget so the next ib's ctx start never stalls
  the PE.
- PSUM->SBUF evacuations alternate between DVE and gpsimd (Pool) to balance
  engine load; all matmuls bf16 with fp32 PSUM accumulation.
"""

import sys

sys.path.insert(0, "/opt/trn_rl_repo")

import numpy as np
import ml_dtypes

import concourse.bacc as bacc
import concourse.tile as tile
import concourse.mybir as mybir
from concourse.bass_utils import run_bass_kernel_spmd

F32 = mybir.dt.float32
BF16 = mybir.dt.bfloat16
F16 = mybir.dt.float16
MMDT = BF16
MM_NP = ml_dtypes.bfloat16
AF = mybir.ActivationFunctionType

B, T, D, ATT = 4, 2048, 1024, 1024
NH, DH = 16, 64
CD = 128  # att-dim slice per core (2 heads)
NCORES = 8
LARGE = 1e30
SW = 512  # time-span width per streamed input chunk

_cache = {}


def build_nc(NJS, phases="ABC", loop_n=1, splice=True, ebufs=2, pcybufs=2,
             ppabufs=2, bcast="gpsimd", expbufs=6, pair_budget=800,
             bound_budget=3000, y_split=4, fastrecip=True, order="seq",
             abl=()):
    """NJS: tuple of per-batch NJ (Tv tiles of 128), in BATCH order."""
    NJS = tuple(int(x) for x in NJS)
    key = (NJS, phases, loop_n, splice, ebufs, pcybufs, ppabufs, bcast,
           expbufs, pair_budget, bound_budget, y_split, fastrecip, order,
           tuple(abl))
    if key in _cache:
        return _cache[key]
    NSV = [(nj + 3) // 4 for nj in NJS]  # 512-spans of Tv per batch
    TV = [s * SW for s in NSV]
    OFF = np.cumsum([0] + TV).tolist()  # kT/xv col offsets per batch
    JOFF = np.cumsum([0] + list(NJS)).tolist()  # v/mask tile offsets
    TVT, NJT = OFF[B], JOFF[B]
    # process batches in descending NJ (ties by index); keep the smallest
    # batch second-to-last so the final batch retains attention slack
    border = sorted(range(B), key=lambda b: -NJS[b])
    if B >= 2:
        border = border[:-2] + [border[-1], border[-2]]

    nc = bacc.Bacc("TRN2", target_bir_lowering=False, debug=False,
                   num_devices=NCORES)

    xq_d = nc.dram_tensor("xq", [B, D, T], MMDT, kind="ExternalInput")
    xv_d = nc.dram_tensor("xv", [D, TVT], MMDT, kind="ExternalInput")
    wq_d = nc.dram_tensor("wq", [D, CD], MMDT, kind="ExternalInput")
    wk_d = nc.dram_tensor("wk", [D, CD], MMDT, kind="ExternalInput")
    wv_d = nc.dram_tensor("wv", [D, 130], MMDT, kind="ExternalInput")
    wf_d = nc.dram_tensor("wf", [CD, ATT], MMDT, kind="ExternalInput")
    mask_d = nc.dram_tensor("mask", [128, NJT], F32, kind="ExternalInput")
    bqk_d = nc.dram_tensor("bqk", [128, 2 * B], F32, kind="ExternalInput")
    y_d = nc.dram_tensor("y", [B, T, ATT], F16, kind="ExternalOutput")

    xv_r = xv_d[:, :].rearrange("(kc p) n -> p kc n", p=128)  # [128, 8, TVT]
    wq_r = wq_d[:, :].rearrange("(kc p) m -> p kc m", p=128)  # [128, 8, 128]
    wk_r = wk_d[:, :].rearrange("(kc p) m -> p kc m", p=128)
    wv_r = wv_d[:, :].rearrange("(kc p) m -> p kc m", p=128)  # [128, 8, 130]

    with tile.TileContext(nc) as tc:
        from contextlib import ExitStack
        _st = ExitStack()
        if loop_n > 1:
            _st.enter_context(tc.For_i(0, loop_n, 1))
        with _st, tc.tile_pool(name="persist", bufs=1) as persist:
            qT = persist.tile([128, B, T], MMDT)
            kT = persist.tile([128, TVT], MMDT)
            v = persist.tile([128, NJT, 130], MMDT)
            ctxT = persist.tile([128, B, T], MMDT)
            wfs = persist.tile([128, ATT], MMDT)
            wqs = persist.tile([128, 8, CD], MMDT)
            wks = persist.tile([128, 8, CD], MMDT)
            wvs = persist.tile([128, 8, 130], MMDT)
            mask = persist.tile([128, NJT], F32)
            bqk = persist.tile([128, 2 * B], F32)

            with (
                tc.tile_pool(name="chunks", bufs=3) as chunks,
                tc.tile_pool(name="expp", bufs=expbufs) as expp,
                tc.tile_pool(name="workp", bufs=4) as workp,
                tc.tile_pool(name="yp", bufs=4) as yp,
                tc.tile_pool(name="rsd", bufs=4, space="DRAM") as rsd,
                tc.tile_pool(name="ppa", bufs=ppabufs, space="PSUM") as ppa,
                tc.tile_pool(name="pe", bufs=ebufs, space="PSUM") as pe_pool,
                tc.tile_pool(name="pcy", bufs=pcybufs, space="PSUM") as pcy,
            ):
                # ---------- filler units (emitted into attention slack) ----
                fillers = []  # list of (kind, est_ns, closure)

                def pop_fillers(budget_ns):
                    while fillers and budget_ns > 0:
                        _, est, fn = fillers.pop(0)
                        fn()
                        budget_ns -= est

                def drain_matching(pred):
                    # emission-order defines the dependency graph: any write
                    # that attention reads must be emitted before the reader
                    while any(pred(k) for k, _, _ in fillers):
                        _, _, fn = fillers.pop(0)
                        fn()

                # ---------- projection unit builders ----------------------
                def dma_weights():
                    # issue from the otherwise-idle SP queue: a dma_start on
                    # the scalar queue costs the ACT sequencer ~667ns each,
                    # which collides with the exp stream at loop boundaries
                    # mask first: the next iteration's first exp reads it, so
                    # keep it ahead of the bulk weights on the sync DGE ring
                    nc.sync.dma_start(out=mask, in_=mask_d[:, :])
                    nc.sync.dma_start(out=bqk, in_=bqk_d[:, :])
                    for kc in range(8):
                        nc.sync.dma_start(out=wks[:, kc, :], in_=wk_r[:, kc, :])
                    for kc in range(8):
                        nc.sync.dma_start(out=wvs[:, kc, :], in_=wv_r[:, kc, :])
                    for kc in range(8):
                        nc.sync.dma_start(out=wqs[:, kc, :], in_=wq_r[:, kc, :])
                    nc.sync.dma_start(
                        out=wfs,
                        in_=wf_d[:, :].rearrange("(kc p) n -> p kc n", p=128
                                                 )[:, 0, :])
                    # ones columns of v (persist; projection copies skip them)
                    vh = v[:, :, :].rearrange("p j (h x) -> p j h x", h=2, x=65)
                    nc.vector.memset(vh[:, :, :, 64:65], 1.0)

                def emit_xv_chunk(b, s, split=False):
                    xc = chunks.tile([128, 8, SW], MMDT, tag="xc",
                                     name=f"xv_{b}_{s}")
                    sl = slice(OFF[b] + s * SW, OFF[b] + (s + 1) * SW)
                    if split:
                        # halve time-to-first-matmul: two queues in parallel
                        nc.sync.dma_start(out=xc[:, 0:4, :],
                                          in_=xv_r[:, 0:4, sl])
                        nc.gpsimd.dma_start(out=xc[:, 4:8, :],
                                            in_=xv_r[:, 4:8, sl])
                    else:
                        nc.sync.dma_start(out=xc, in_=xv_r[:, :, sl])
                    return xc

                def emit_xq_chunk(b, s):
                    xc = chunks.tile([128, 8, SW], MMDT, tag="xc",
                                     name=f"xq_{b}_{s}")
                    xq_rb = xq_d[b, :, :].rearrange("(kc p) n -> p kc n", p=128)
                    nc.gpsimd.dma_start(out=xc,
                                        in_=xq_rb[:, :, s * SW:(s + 1) * SW])
                    return xc

                def emit_qk_half(xc, ps_cell, wt, dst, bias_col, tag, half):
                    # one K- or Q-projection span split in two 4-mm halves
                    # sharing a single PSUM accumulation group
                    if half == 0:
                        ps_cell[0] = ppa.tile([128, SW], F32, tag="pa",
                                              name=tag)
                    ps = ps_cell[0]
                    for kc in range(half * 4, half * 4 + 4):
                        nc.tensor.matmul(ps[:, :], lhsT=wt[:, kc, :],
                                         rhs=xc[:, kc, :],
                                         start=(kc == 0), stop=(kc == 7))
                    if half == 1:
                        with nc.allow_low_precision(reason="qk store"):
                            nc.vector.tensor_scalar_add(dst, ps[:, :],
                                                        bias_col)

                def emit_v_unit(xc, b, s, jt):
                    j = s * 4 + jt
                    if j >= NJS[b]:
                        return
                    ps = ppa.tile([128, 512], F32, tag="pa", name=f"v_{b}_{j}")
                    for kc in range(8):
                        nc.tensor.matmul(ps[:, 0:130],
                                         lhsT=xc[:, kc, jt * 128:(jt + 1) * 128],
                                         rhs=wvs[:, kc, :],
                                         start=(kc == 0), stop=(kc == 7))
                    # copy the two 64-wide head slices, skip the ones columns
                    psh = ps[:, 0:130].rearrange("p (h x) -> p h x", x=65)
                    vj = v[:, JOFF[b] + j, :].rearrange("p (h x) -> p h x", x=65)
                    with nc.allow_low_precision(reason="v store"):
                        nc.vector.tensor_copy(out=vj[:, :, 0:64],
                                              in_=psh[:, :, 0:64])

                def proj_units(b, split0=False):
                    """Filler units for batch b's projections, in dep order."""
                    units = []
                    for s in range(NSV[b]):
                        kind = ("projkv", b, s)
                        xc = [None]

                        def load(b=b, s=s, xc=xc, sp=(split0 and s == 0)):
                            xc[0] = emit_xv_chunk(b, s, split=sp)
                        units.append((kind, 150, load))
                        kps = [None]
                        ksl = slice(OFF[b] + s * SW, OFF[b] + (s + 1) * SW)
                        for half in range(2):
                            units.append(
                                (kind, 900, lambda xc=xc, kps=kps,
                                 b=b, s=s, h=half, ksl=ksl:
                                 emit_qk_half(xc[0], kps, wks, kT[:, ksl],
                                              bqk[:, B + b:B + b + 1],
                                              f"k_{b}_{s}", h)))
                        for jt in range(4):
                            units.append((kind, 470,
                                          lambda b=b, s=s, jt=jt, xc=xc:
                                          emit_v_unit(xc[0], b, s, jt)))
                    for s in range(4):
                        qkind = ("projq", b, s)
                        xc = [None]

                        def load(b=b, s=s, xc=xc):
                            xc[0] = emit_xq_chunk(b, s)
                        units.append((qkind, 150, load))
                        qps = [None]
                        for half in range(2):
                            units.append(
                                (qkind, 900, lambda xc=xc, qps=qps,
                                 b=b, s=s, h=half:
                                 emit_qk_half(xc[0], qps, wqs,
                                              qT[:, b, s * SW:(s + 1) * SW],
                                              bqk[:, b:b + 1],
                                              f"q_{b}_{s}", h)))
                    return units

                y_cell = [None]
                evac_ctr = [0]

                def emit_c_unit(b, i, n):
                    # 8 units (4 i-tiles x 2 n-halves) share one staging tile;
                    # the last unit of the (b, ib) block ships a single DMA.
                    q = i % 4
                    if q == 0 and n == 0:
                        y_cell[0] = yp.tile([128, 4, ATT], F16, tag="ysb",
                                            name=f"y_{b}_{i // 4}")
                    y_sb = y_cell[0]
                    y_ps = ppa.tile([128, 512], F32, tag="pa",
                                    name=f"y_{b}_{i}_{n}")
                    nc.tensor.matmul(y_ps[:, :],
                                     lhsT=ctxT[:, b, i * 128:(i + 1) * 128],
                                     rhs=wfs[:, n * 512:(n + 1) * 512],
                                     start=True, stop=True)
                    # gpsimd can't read PSUM: split y evac DVE/ACT to balance
                    evac_ctr[0] += 1
                    with nc.allow_low_precision(reason="y f16"):
                        if "no_yevac" in abl:
                            # ablation: evacuate a sliver only
                            nc.vector.tensor_copy(
                                out=y_sb[:, q, n * 512:n * 512 + 32],
                                in_=y_ps[:, 0:32])
                        elif y_split and evac_ctr[0] % y_split == 0:
                            nc.scalar.copy(
                                out=y_sb[:, q, n * 512:(n + 1) * 512],
                                in_=y_ps[:, :])
                        else:
                            nc.vector.tensor_copy(
                                out=y_sb[:, q, n * 512:(n + 1) * 512],
                                in_=y_ps[:, :])
                    if q == 3 and n == 1:
                        ib = i // 4
                        dst = y_d[b, ib * 512:(ib + 1) * 512, :].rearrange(
                            "(q p) n -> p q n", p=128)
                        nc.sync.dma_start(out=dst, in_=y_sb[:, :, :])

                # ---------- emission ---------------------------------------
                dma_weights()
                if "A" in phases:
                    # inline only what ib=0/j=0 needs: first batch's first
                    # K/V span + first Q span; later spans drain just-in-time
                    b0 = border[0]
                    units0 = proj_units(b0, split0=True)
                    nkv = 7 * NSV[b0]  # 7 units per K/V span
                    inline0 = units0[0:7] + units0[nkv:nkv + 3]
                    for _, _, fn in inline0:
                        fn()
                    fillers.extend(units0[7:nkv] + units0[nkv + 3:])

                # block order: "seq" = all 4 ibs of each batch in border
                # order; "alt" = alternate big/small batches so the small
                # batches' softmax chains hide under big batches' exp streams
                if order == "alt" and B == 4:
                    hi1, hi2, lo1, lo2 = (border[0], border[1], border[3],
                                          border[2])
                    blocks = []
                    for ib in range(4):
                        blocks.append((hi1, ib))
                        blocks.append((lo2, ib))
                    for ib in range(4):
                        blocks.append((hi2, ib))
                        blocks.append((lo1, ib))
                    appear = [hi1, lo2, hi2, lo1]
                else:
                    blocks = [(b, ib) for b in border for ib in range(4)]
                    appear = list(border)
                extended = {appear[0]}
                for blki, (b, ib) in enumerate(
                        blocks if "B" in phases else []):
                    # keep one batch of projection fillers ahead of need
                    ai = appear.index(b)
                    for nb in appear[:min(ai + 2, B)]:
                        if nb not in extended and "A" in phases:
                            extended.add(nb)
                            fillers.extend(proj_units(nb))
                            if not splice:
                                pop_fillers(1e9)
                    NJ = NJS[b]
                    if True:
                        drain_matching(
                            lambda k: k[0] == "projq" and k[1] == b
                            and k[2] <= ib)
                        if blki > 0:
                            drain_matching(
                                lambda k: k[0] == "projkv" and k[1] == b)
                        ibs = slice(ib * 512, (ib + 1) * 512)
                        ctxA = pcy.tile([65, 512], F32, tag="cy")
                        ctxB = pcy.tile([65, 512], F32, tag="cy")
                        ctx_ps = (ctxA[:, :], ctxB[:, :])
                        jlist = list(range(NJ))
                        pairs = [jlist[i:i + 2] for i in range(0, NJ, 2)]
                        bi0 = (blki == 0)

                        def emit_ctx(pend, b=b, NJ=NJ, ctx_ps=ctx_ps):
                            for j, ex in pend:
                                for hh in range(2):
                                    nc.tensor.matmul(
                                        ctx_ps[hh],
                                        lhsT=v[:, JOFF[b] + j,
                                               hh * 65:(hh + 1) * 65],
                                        rhs=ex[:, hh * 512:(hh + 1) * 512],
                                        start=(j == 0), stop=(j == NJ - 1),
                                    )

                        pending = None
                        for jp in pairs:
                            if bi0:
                                drain_matching(
                                    lambda k: k[0] == "projkv" and k[1] == b
                                    and k[2] <= jp[-1] // 4)
                            exs = []
                            for j in jp:
                                e_ps = pe_pool.tile([128, 1024], F32, tag="e")
                                jsl = slice(OFF[b] + j * 128,
                                            OFF[b] + (j + 1) * 128)
                                for hh in range(2):
                                    hp = slice(hh * 64, (hh + 1) * 64)
                                    nc.tensor.matmul(
                                        e_ps[:, hh * 512:(hh + 1) * 512],
                                        lhsT=kT[hp, jsl],
                                        rhs=qT[hp, b, ibs],
                                        start=True, stop=True,
                                    )
                                ex = expp.tile([128, 1024], MMDT, tag="ex")
                                if "exp_small" in abl:
                                    # ablation: tiny ACT op (garbage numerics)
                                    nc.scalar.activation(
                                        out=ex[:, 0:64], in_=e_ps[:, 0:64],
                                        func=AF.Exp,
                                        bias=mask[:, JOFF[b] + j:
                                                  JOFF[b] + j + 1],
                                        scale=1.0)
                                else:
                                    nc.scalar.activation(
                                        out=ex[:, :], in_=e_ps[:, :],
                                        func=AF.Exp,
                                        bias=mask[:, JOFF[b] + j:
                                                  JOFF[b] + j + 1],
                                        scale=1.0)
                                exs.append((j, ex))
                            if splice:
                                pop_fillers(pair_budget * len(jp))
                            if pending is not None:
                                emit_ctx(pending)
                            pending = exs
                        if pending is not None:
                            emit_ctx(pending)
                        for hh in range(2):
                            p0 = hh * 64
                            if "no_chain" in abl:
                                with nc.allow_low_precision(reason="cs"):
                                    nc.vector.tensor_copy(
                                        out=ctxT[p0:p0 + 64, b, ibs],
                                        in_=ctx_ps[hh][0:64, :])
                                continue
                            rs = workp.tile([1, 512], F32, tag="rs")
                            if fastrecip:
                                # the custom-DVE approx op misreads PSUM
                                # sources (validated on HW): stage the psum
                                # denominator row to SBUF first
                                den = workp.tile([1, 512], F32, tag="den")
                                nc.vector.tensor_copy(
                                    out=den[:, :], in_=ctx_ps[hh][64:65, :])
                                nc.vector.reciprocal_approx_fast(
                                    out=rs[:, :], in_=den[:, :])
                            else:
                                nc.vector.reciprocal(
                                    out=rs[:, :], in_=ctx_ps[hh][64:65, :])
                            bc_sb = workp.tile([64, 512], F32, tag="bcs")
                            if bcast == "gpsimd":
                                nc.gpsimd.partition_broadcast(
                                    bc_sb[:, :], rs[:, :], channels=64)
                            else:
                                rs_dr = rsd.tile([1, 512], F32, tag="rsd")
                                nc.sync.dma_start(out=rs_dr[:, :], in_=rs[:, :])
                                nc.sync.dma_start(
                                    out=bc_sb[:, :],
                                    in_=rs_dr[0:1, :].partition_broadcast(64))
                            with nc.allow_low_precision(reason="ctx store"):
                                nc.vector.tensor_mul(
                                    ctxT[p0:p0 + 64, b, ibs],
                                    ctx_ps[hh][0:64, :], bc_sb[:, :])
                        if splice:
                            pop_fillers(bound_budget)
                        if "C" in phases:
                            for i in range(ib * 4, ib * 4 + 4):
                                for n in range(2):
                                    fillers.append(
                                        ("c", 400, lambda b=b, i=i, n=n:
                                         emit_c_unit(b, i, n)))
                        if not splice:
                            pop_fillers(1e9)
                if "A" in phases:
                    for nb in appear:
                        if nb not in extended:
                            extended.add(nb)
                            fillers.extend(proj_units(nb))
                pop_fillers(1e9)
    nc.compile()
    _cache[key] = nc
    return nc


def make_in_maps(query, value, value_lens, Wq, bq, Wk, bk, Wv, bv, Wf, bf):
    query = np.ascontiguousarray(np.asarray(query, np.float32))
    value = np.ascontiguousarray(np.asarray(value, np.float32))
    value_lens = np.asarray(value_lens)
    Wq = np.asarray(Wq, np.float32)
    Wk = np.asarray(Wk, np.float32)
    Wv = np.asarray(Wv, np.float32)
    Wf = np.asarray(Wf, np.float32)
    bq = np.asarray(bq, np.float32)
    bk = np.asarray(bk, np.float32)

    scale = np.float32(1.0 / np.sqrt(np.float32(DH)))
    effL = [int(l) if l > 0 else T for l in value_lens]
    NJS = tuple(max(1, int(np.ceil(l / 128))) for l in effL)
    NSV = [(nj + 3) // 4 for nj in NJS]
    TV = [s * SW for s in NSV]
    NJT = sum(NJS)

    mask = np.zeros((128, NJT), np.float32)
    joff = 0
    for b in range(B):
        L = int(value_lens[b])
        if L > 0:
            idx = np.arange(NJS[b] * 128).reshape(NJS[b], 128).T  # [128, NJ]
            mb = np.zeros((128, NJS[b]), np.float32)
            mb[idx >= L] = -LARGE
            mask[:, joff:joff + NJS[b]] = mb
        joff += NJS[b]

    xq = np.empty((B, D, T), MM_NP)
    for b in range(B):
        xq[b] = 0 if int(value_lens[b]) == 0 else query[b].T.astype(MM_NP)
    xv = np.concatenate(
        [value[b].T[:, :TV[b]].astype(MM_NP) for b in range(B)], axis=1)
    xv = np.ascontiguousarray(xv)

    in_maps = []
    for c in range(NCORES):
        cs = slice(c * CD, (c + 1) * CD)
        wq = (Wq[:, cs] * scale).astype(MM_NP)
        wk = Wk[:, cs].astype(MM_NP)
        wv = np.zeros((D, 130), np.float32)
        for h in range(2):
            wv[:, h * 65:h * 65 + 64] = Wv[:, c * CD + h * 64:
                                           c * CD + (h + 1) * 64]
        wf = Wf[cs, :].astype(MM_NP)
        bqk = np.zeros((128, 2 * B), np.float32)
        for b in range(B):
            if int(value_lens[b]) != 0:
                bqk[:, b] = bq[cs] * scale
            bqk[:, B + b] = bk[cs]
        in_maps.append({
            "xq": xq, "xv": xv,
            "wq": wq, "wk": wk, "wv": wv.astype(MM_NP), "wf": wf,
            "mask": mask, "bqk": bqk,
        })
    return in_maps, NJS


def assemble(results, Wv, bv, Wf, bf):
    bv = np.asarray(bv, np.float32)
    Wf = np.asarray(Wf, np.float32)
    bf = np.asarray(bf, np.float32)
    const = (bv @ Wf + bf).astype(np.float32)
    acc = np.zeros((B, T, ATT), np.float32)
    for r in results:
        acc += r["y"].astype(np.float32)
    return acc + const


def kernel(query, value, value_lens, Wq, bq, Wk, bk, Wv, bv, Wf, bf):
    in_maps, NJS = make_in_maps(query, value, value_lens, Wq, bq, Wk, bk,
                                Wv, bv, Wf, bf)
    nc = build_nc(NJS)
    res = run_bass_kernel_spmd(nc, in_maps, list(range(NCORES)))
    return assemble(res.results, Wv, bv, Wf, bf)
